# revision 42
# baseline (speedup 1.0000x reference)
"""TRN2 Bass kernel for nn_AttentionBlock (N=4, C=256, L=4096, 4 heads, AGGR=4).

Sharding: 8 cores = (batch n, L-half). Core c handles n=c//2, query positions
l in [half*2048, (half+1)*2048). Each core computes k/v from the full
aggregated sequence of its batch (L2=1024) and produces the full output slice
out[n][:, l_half] -- no cross-core reduction needed.

The host hands each core x[n] with columns PERMUTED so the core's own query
half comes first (attention is permutation-invariant over key positions, and
the 4-wide pooling windows stay intact), so the query slice is a static
[:, 0:2048] view and is available as soon as the first DMA half lands.

Cost-model shape: every engine instruction costs (free-dim cols) x cycle_t;
PE matmuls cost (out free cols) x 0.417ns regardless of contraction width.
The kernel is PE-bound (~70us of matmul cols), so softmax exp -- the other
big consumer (64 tiles x 1024 cols) -- is split across three engines so none
exceeds PE: ACT runs real Exp; Pool and DVE run a Schraudolph bit-trick exp
(i32 = trunc(S*2^23*log2e*0.125 + bias), bitcast as f32 ~ exp(S/8) within
3%), writing int32 tiles the o-matmul consumes as float32r. f32->f32r
bitcast views avoid all weight/x re-typing copies.
"""

import numpy as np

N, C, L = 4, 256, 4096
HEAD_DIM = 64
H = C // HEAD_DIM          # 4 heads
AGGR = 4
L2 = L // AGGR             # 1024 aggregated positions
LH = L // 2                # 2048 query positions per core
BN_EPS = 1e-5
N_CORES = 8

# Schraudolph exp-trick constants, int16/bfloat16 variant (trunc/floor):
# i16 = trunc(S * 2^7*log2e/8 + (127*2^7 - bias)); i16 bits read as bf16
# give exp(S/8) within ~3%. Folds the 1/sqrt(E)=1/8 score scale.
TRICK_A = 128.0 * 1.4426950408889634 * 0.125
TRICK_B = float(127 << 7) - 366400.0 / 65536.0

_CACHE = {}


def _build_program():
    import concourse.bass as bass
    import concourse.bacc as bacc
    import concourse.tile as tile
    from concourse import mybir
    from contextlib import ExitStack

    dt = mybir.dt
    f32 = dt.float32
    f32r = dt.float32r
    bf16 = dt.bfloat16
    i16 = dt.int16
    AF = mybir.ActivationFunctionType
    Alu = mybir.AluOpType

    nc = bacc.Bacc("TRN2", debug=False, num_devices=N_CORES)

    xf_d = nc.dram_tensor("x_full", [C, L], bf16, kind="ExternalInput")
    wqt_d = nc.dram_tensor("wqt", [C, C], bf16, kind="ExternalInput")
    wkt_d = nc.dram_tensor("wkt", [C, C], bf16, kind="ExternalInput")
    wvt_d = nc.dram_tensor("wvt", [C, C], bf16, kind="ExternalInput")
    wot_d = nc.dram_tensor("wot", [C, C], bf16, kind="ExternalInput")
    wat_d = nc.dram_tensor("wat", [C, C], bf16, kind="ExternalInput")
    # rows: bq, bk, t(bn-folded xa bias), bo
    bp_d = nc.dram_tensor("biasp", [4, C], f32, kind="ExternalInput")
    bv_d = nc.dram_tensor("bv", [C], f32, kind="ExternalInput")
    out_d = nc.dram_tensor("out", [C, LH], f32, kind="ExternalOutput")

    # Each exp chunk is split by columns: ACT runs real Exp on the first
    # EXP_ACT_COLS, DVE runs the int16 Schraudolph trick on the rest, in
    # parallel -- chunk latency ~0.71us < PE's 0.85us per-chunk appetite,
    # so PE (not the exp engines) paces the pipeline.
    EXP_ACT_COLS = 768

    with tile.TileContext(nc) as tc, ExitStack() as ctx:
        pp = ctx.enter_context(tc.tile_pool(name="persist", bufs=1))
        scr_w = ctx.enter_context(tc.tile_pool(name="scr_w", bufs=1))
        scr_p = ctx.enter_context(tc.tile_pool(name="scr_p", bufs=3))
        at_pool = ctx.enter_context(tc.tile_pool(name="at", bufs=6))
        oa_pool = ctx.enter_context(tc.tile_pool(name="oa", bufs=2))
        outp = ctx.enter_context(tc.tile_pool(name="outp", bufs=3))
        r_pool = ctx.enter_context(tc.tile_pool(name="rp", bufs=2))
        R_pool = ctx.enter_context(tc.tile_pool(name="Rp", bufs=3))

        ps_s = ctx.enter_context(tc.tile_pool(name="ps_s", bufs=3, space="PSUM"))
        ps_o = ctx.enter_context(tc.tile_pool(name="ps_o", bufs=2, space="PSUM"))

        # ---- persistent tiles ----
        xf = [pp.tile([128, L], bf16, name=f"xf{ct}", tag=f"xf{ct}")
              for ct in range(2)]
        q_r = [pp.tile([128, LH], bf16, name=f"qr{ct}", tag=f"qr{ct}")
               for ct in range(2)]
        k_r = [pp.tile([128, L2], bf16, name=f"kr{ct}", tag=f"kr{ct}")
               for ct in range(2)]
        xa_r = [pp.tile([128, L2], bf16, name=f"xar{ct}", tag=f"xar{ct}")
                for ct in range(2)]
        p_r = [pp.tile([128, L2], bf16, name=f"pr{ct}", tag=f"pr{ct}")
               for ct in range(2)]
        # v'^T per m-tile: 4 heads x (64 cols + ones col), bf16
        v_r = [pp.tile([128, 4 * 65], bf16, name=f"vr{mt}", tag=f"vr{mt}")
               for mt in range(8)]
        bias_t = [pp.tile([128, 4], f32, name=f"bias{ct}", tag=f"bias{ct}")
                  for ct in range(2)]
        bvb = pp.tile([128, C], f32, name="bvb", tag="bvb")

        # ---- DMAs: the cost model serializes DMA transfers (~360 B/ns
        # aggregate), so everything goes on the SP queue in first-use order;
        # putting DMAs on the ACT queue head-of-line-blocks its sequencer.
        wt_dram = {"wqt": wqt_d, "wkt": wkt_d, "wvt": wvt_d, "wot": wot_d,
                   "wat": wat_d}
        w_f = {}

        def w_dma(wname):
            wf = scr_w.tile([128, 512], bf16, name=f"wf_{wname}",
                            tag=f"wf_{wname}")
            src = wt_dram[wname].ap().rearrange("(k p) o -> p k o", p=128)
            nc.sync.dma_start(wf[:].rearrange("p (k o) -> p k o", k=2), src)
            w_f[wname] = wf

        def x_dma(half, sub):
            for ct in range(2):
                c0 = half * 2048 + sub * 1024
                nc.sync.dma_start(
                    xf[ct][:, c0:c0 + 1024],
                    xf_d.ap()[ct * 128:(ct + 1) * 128, c0:c0 + 1024])

        for ct in range(2):
            nc.sync.dma_start(
                bias_t[ct][:], bp_d.ap().rearrange("b (k p) -> k p b", p=128)[ct])
        bv_f = r_pool.tile([1, C], f32, name="bv_f", tag="bv_f", bufs=1)
        nc.sync.dma_start(bv_f[:], bv_d.ap().rearrange("(a o) -> a o", a=1))
        w_dma("wqt")
        x_dma(0, 0)   # cols 0:1024, both ct tiles
        w_dma("wat")
        w_dma("wkt")
        w_dma("wvt")
        x_dma(0, 1)
        x_dma(1, 0)
        x_dma(1, 1)
        w_dma("wot")

        # ---- constants ----
        for mt in range(8):
            nc.gpsimd.memset(
                v_r[mt][:].rearrange("p (h e) -> p h e", e=65)[:, :, 64], 1.0)
        # pre-warm the ACT exp table during the idle prefix
        warm = scr_w.tile([1, 8], f32, name="warm", tag="warm")
        ones_f = scr_w.tile([1, 8], f32, name="ones_f", tag="ones_f")
        nc.gpsimd.memset(ones_f[:], 1.0)
        nc.scalar.activation(warm[:], ones_f[:], AF.Exp, scale=1.0)
        # bv broadcast to all partitions (for the v-drain fused bias)
        nc.gpsimd.partition_broadcast(bvb[:], bv_f[:], channels=128)

        def w_block(wname, cch, ct_out):
            # lhsT block [c_in 128, c_out 128] for chunk cch, out tile ct_out
            return w_f[wname][:, cch * 256 + ct_out * 128:
                              cch * 256 + ct_out * 128 + 128]

        # ---- pool quadrants: p = avg4 + max4 ----
        def pool_quadrant(mc, ct, eng, sub=None):
            c0, cw = mc * 2048, 2048
            s0, sw = mc * 512, 512
            if sub is not None:
                c0, cw = c0 + sub * 1024, 1024
                s0, sw = s0 + sub * 256, 256
            xv = xf[ct][:, c0:c0 + cw].rearrange("p (m g) -> p m g", g=4)
            a1 = scr_p.tile([128, 512], f32, name="pa1", tag="pa1")
            a2 = scr_p.tile([128, 512], f32, name="pa2", tag="pa2")
            m1 = scr_p.tile([128, 512], f32, name="pm1", tag="pm1")
            m2 = scr_p.tile([128, 512], f32, name="pm2", tag="pm2")
            eng.tensor_tensor(a1[:, 0:sw], xv[:, :, 0], xv[:, :, 1], Alu.add)
            eng.tensor_tensor(a2[:, 0:sw], xv[:, :, 2], xv[:, :, 3], Alu.add)
            eng.tensor_tensor(m1[:, 0:sw], xv[:, :, 0], xv[:, :, 1], Alu.max)
            eng.tensor_tensor(m2[:, 0:sw], xv[:, :, 2], xv[:, :, 3], Alu.max)
            eng.tensor_tensor(a1[:, 0:sw], a1[:, 0:sw], a2[:, 0:sw], Alu.add)
            eng.tensor_tensor(m1[:, 0:sw], m1[:, 0:sw], m2[:, 0:sw], Alu.max)
            eng.scalar_tensor_tensor(
                p_r[ct][:, s0:s0 + sw], a1[:, 0:sw], 0.25, m1[:, 0:sw],
                Alu.mult, Alu.add)

        # ---- projection chunk helpers ----
        def proj_chunk(wname, src, dst, bias_col, c0, cw, eng):
            for ct_out in range(2):
                ps = ps_s.tile([128, cw], f32, name="ps_s", tag="ps_s")
                for cch in range(2):
                    nc.tensor.matmul(
                        ps[:], w_block(wname, cch, ct_out),
                        src[cch][:, c0:c0 + cw],
                        start=(cch == 0), stop=(cch == 1))
                if eng is nc.scalar:
                    nc.scalar.add(dst[ct_out][:, c0:c0 + cw],
                                  ps[:], bias_t[ct_out][:, bias_col:bias_col + 1])
                else:
                    eng.tensor_scalar(
                        dst[ct_out][:, c0:c0 + cw], ps[:],
                        bias_t[ct_out][:, bias_col:bias_col + 1], None, Alu.add)

        def q_chunk(lcq, eng):
            for ct_out in range(2):
                ps = ps_s.tile([128, 512], f32, name="ps_s", tag="ps_s")
                for cch in range(2):
                    nc.tensor.matmul(
                        ps[:], w_block("wqt", cch, ct_out),
                        xf[cch][:, lcq * 512:(lcq + 1) * 512],
                        start=(cch == 0), stop=(cch == 1))
                if eng is nc.scalar:
                    nc.scalar.add(q_r[ct_out][:, lcq * 512:(lcq + 1) * 512],
                                  ps[:], bias_t[ct_out][:, 0:1])
                else:
                    eng.tensor_scalar(
                        q_r[ct_out][:, lcq * 512:(lcq + 1) * 512], ps[:],
                        bias_t[ct_out][:, 0:1], None, Alu.add)

        def v_block(mt, drain_eng, vpool=None):
            vpool = vpool or ps_o
            tag = "ps_o" if vpool is ps_o else "ps_s"
            pv = vpool.tile([128, C], f32, name="ps_v", tag=tag)
            for cch in range(2):
                nc.tensor.matmul(
                    pv[:], xa_r[cch][:, mt * 128:(mt + 1) * 128],
                    w_f["wvt"][:, cch * 256:(cch + 1) * 256],
                    start=(cch == 0), stop=(cch == 1))
            vv = v_r[mt][:].rearrange("p (h e) -> p h e", e=65)
            # fused +bv via the broadcast bias tile
            drain_eng.scalar_tensor_tensor(
                vv[:, :, 0:64], pv[:].rearrange("p (h e) -> p h e", e=64),
                1.0, bvb[:].rearrange("p (h e) -> p h e", e=64),
                Alu.mult, Alu.add)

        # ---- key-block groups: 256 keys each, gated on one x DMA pair.
        # pool both ct tiles -> xa block -> k block -> two v blocks ----
        def blkgrp(b, drain_eng):
            mc, sub = divmod(b, 2)
            pool_quadrant(mc, 0, nc.vector, sub=sub)
            pool_quadrant(mc, 1, nc.vector, sub=sub)
            proj_chunk("wat", p_r, xa_r, 2, b * 256, 256, drain_eng)
            proj_chunk("wkt", xa_r, k_r, 1, b * 256, 256, drain_eng)
            for mt in (2 * b, 2 * b + 1):
                v_block(mt, nc.vector, vpool=ps_s)

        # ---- prefix: q0/q1 + first two key-block groups (x half 0) ----
        q_chunk(0, nc.scalar)
        blkgrp(0, nc.scalar)
        q_chunk(1, nc.scalar)
        blkgrp(1, nc.scalar)
        q_chunk(2, nc.vector)
        q_chunk(3, nc.vector)

        # ---- attention: o-matmuls lag exp by one m-tile; the previous
        # iteration's softmax-normalize and Wo conv are emitted inside the
        # next iteration's S/exp stream so they overlap it ----
        oa_tiles = {}

        def norm_prev(state):
            lc, hp, po = state
            oa = oa_tiles[lc]
            for h2 in range(2):
                r_t = r_pool.tile([1, 512], f32, name="r", tag="r")
                nc.vector.reciprocal(r_t[:], po[h2][64:65, :])
                R_t = R_pool.tile([64, 512], f32, name="R", tag="R")
                nc.gpsimd.partition_broadcast(R_t[:], r_t[:], channels=64)
                nc.vector.tensor_tensor(
                    oa[hp][h2 * 64:(h2 + 1) * 64, :], po[h2][0:64, :],
                    R_t[:], Alu.mult)

        def wo_prev(state):
            lc, hp, po = state
            if hp != 1:
                return
            oa = oa_tiles[lc]
            for ct_out in range(2):
                psW = ps_s.tile([128, 512], f32, name="ps_s", tag="ps_s")
                for cch in range(2):
                    nc.tensor.matmul(
                        psW[:], w_block("wot", cch, ct_out), oa[cch][:],
                        start=(cch == 0), stop=(cch == 1))
                out_t = outp.tile([128, 512], f32, name="out", tag="out")
                nc.scalar.add(out_t[:], psW[:], bias_t[ct_out][:, 3:4])
                nc.sync.dma_start(
                    out_d.ap()[ct_out * 128:(ct_out + 1) * 128,
                               lc * 512:(lc + 1) * 512], out_t[:])
            del oa_tiles[lc]

        # pending o-matmul FIFO: one pair popped per (S, exp) step, crossing
        # iteration boundaries so PE never waits on the last exp of an iter
        pending = []
        it_idx = [0]

        def emit_iter(lc, hp, prev_state, hooks=None):
            it = it_idx[0]
            it_idx[0] += 1
            if hp == 0:
                oa_tiles[lc] = [
                    oa_pool.tile([128, 512], bf16, name=f"oa{ct}",
                                 tag=f"oa{ct}") for ct in range(2)]
            po = [ps_o.tile([65, 512], f32, name="ps_o", tag="ps_o")
                  for _ in range(2)]

            def make_o(mt, at_ap):
                def emit():
                    for h2 in range(2):
                        h = 2 * hp + h2
                        nc.tensor.matmul(
                            po[h2][:], v_r[mt][:, h * 65:h * 65 + 65],
                            at_ap[:, h2 * 512:(h2 + 1) * 512],
                            start=(mt == 0), stop=(mt == 7))
                return emit

            for mt in range(8):
                if hooks and mt in hooks:
                    hooks[mt]()
                ps = ps_s.tile([128, L2], f32, name="ps_s", tag="ps_s")
                for h2 in range(2):
                    nc.tensor.matmul(
                        ps[:, h2 * 512:(h2 + 1) * 512],
                        k_r[hp][h2 * 64:(h2 + 1) * 64, mt * 128:(mt + 1) * 128],
                        q_r[hp][h2 * 64:(h2 + 1) * 64, lc * 512:(lc + 1) * 512],
                        start=True, stop=True)
                ca = EXP_ACT_COLS
                at = at_pool.tile([128, 1024], bf16, name="at", tag="at")
                nc.scalar.activation(at[:, 0:ca], ps[:, 0:ca], AF.Exp,
                                     scale=0.125)
                nc.vector.tensor_scalar(at[:].bitcast(i16)[:, ca:1024],
                                        ps[:, ca:1024], TRICK_A, TRICK_B,
                                        Alu.mult, Alu.add)
                at_ap = at[:]
                pending.append(make_o(mt, at_ap))
                # with the deeper o-FIFO, the previous iteration's last
                # o-matmul is popped during step mt1, so its normalize may
                # be emitted no earlier than mt2 (else it misses mt7)
                if mt == 2 and prev_state is not None:
                    norm_prev(prev_state)
                if mt == 5 and prev_state is not None:
                    wo_prev(prev_state)
                if len(pending) >= 3:
                    pending.pop(0)()
            return (lc, hp, po)

        # iteration (0,0): key-block groups 2/3 (x half 1) stream in mid-iter
        state = emit_iter(0, 0, None, hooks={
            3: lambda: blkgrp(2, nc.vector),
            5: lambda: blkgrp(3, nc.vector),
        })
        for lc, hp in [(0, 1), (1, 0), (1, 1), (2, 0), (2, 1), (3, 0), (3, 1)]:
            state = emit_iter(lc, hp, state)
        while pending:
            pending.pop(0)()
        norm_prev(state)
        wo_prev(state)

    nc.compile()
    return nc


def _get_program():
    if "nc" not in _CACHE:
        _CACHE["nc"] = _build_program()
    return _CACHE["nc"]


def kernel(x, Wq, bq, Wk, bk, Wv, bv, Wo, bo, Wa,
           g1, b1, m1, v1, g2, b2, m2, v2):
    import ml_dtypes
    from concourse import bass_utils

    nc = _get_program()
    bf = ml_dtypes.bfloat16

    x = np.asarray(x, dtype=np.float32).astype(bf)
    # fold both eval-mode BNs into a per-channel affine: xa = s*(Wa@p) + t
    s1 = np.asarray(g1) / np.sqrt(np.asarray(v1) + BN_EPS)
    t1 = np.asarray(b1) - np.asarray(m1) * s1
    s2 = np.asarray(g2) / np.sqrt(np.asarray(v2) + BN_EPS)
    t2 = np.asarray(b2) - np.asarray(m2) * s2
    s = (s1 * s2).astype(np.float32)
    t = (t1 * s2 + t2).astype(np.float32)

    wat = (np.asarray(Wa) * s[:, None]).astype(np.float32).T.astype(bf)
    wqt = np.asarray(Wq, dtype=np.float32).T.astype(bf)
    wkt = np.asarray(Wk, dtype=np.float32).T.astype(bf)
    wvt = np.asarray(Wv, dtype=np.float32).T.astype(bf)
    wot = np.asarray(Wo, dtype=np.float32).T.astype(bf)
    biasp = np.stack([np.asarray(bq), np.asarray(bk), t,
                      np.asarray(bo)]).astype(np.float32)
    bvv = np.asarray(bv, dtype=np.float32)

    shared = {"wqt": wqt, "wkt": wkt, "wvt": wvt, "wot": wot, "wat": wat,
              "biasp": biasp, "bv": bvv}
    in_maps = []
    for c in range(N_CORES):
        n, half = c // 2, c % 2
        m = dict(shared)
        xs = x[n]
        if half == 0:
            m["x_full"] = np.ascontiguousarray(xs)
        else:
            # core's own query half first; key order is irrelevant
            # (pool windows intact, attention permutation-invariant)
            m["x_full"] = np.concatenate([xs[:, LH:], xs[:, :LH]], axis=1)
        in_maps.append(m)

    res = bass_utils.run_bass_kernel_spmd(nc, in_maps,
                                          core_ids=list(range(N_CORES)))
    out = np.empty((N, C, L), np.float32)
    for c in range(N_CORES):
        n, half = c // 2, c % 2
        out[n][:, half * LH:(half + 1) * LH] = res.results[c]["out"]
    return out


# revision 43
# speedup vs baseline: 1.0353x; 1.0353x over previous
"""TRN2 Bass kernel for nn_AttentionBlock (N=4, C=256, L=4096, 4 heads, AGGR=4).

Sharding: 8 cores = (batch n, L-half). Core c handles n=c//2, query positions
l in [half*2048, (half+1)*2048). Each core computes k/v from the full
aggregated sequence of its batch (L2=1024) and produces the full output slice
out[n][:, l_half] -- no cross-core reduction needed.

The host hands each core x[n] with columns PERMUTED so the core's own query
half comes first (attention is permutation-invariant over key positions, and
the 4-wide pooling windows stay intact), so the query slice is a static
[:, 0:2048] view and is available as soon as the first DMA half lands.

Cost-model shape: every engine instruction costs (free-dim cols) x cycle_t;
PE matmuls cost (out free cols) x 0.417ns regardless of contraction width.
The kernel is PE-bound (~70us of matmul cols), so softmax exp -- the other
big consumer (64 tiles x 1024 cols) -- is split across three engines so none
exceeds PE: ACT runs real Exp; Pool and DVE run a Schraudolph bit-trick exp
(i32 = trunc(S*2^23*log2e*0.125 + bias), bitcast as f32 ~ exp(S/8) within
3%), writing int32 tiles the o-matmul consumes as float32r. f32->f32r
bitcast views avoid all weight/x re-typing copies.
"""

import numpy as np

N, C, L = 4, 256, 4096
HEAD_DIM = 64
H = C // HEAD_DIM          # 4 heads
AGGR = 4
L2 = L // AGGR             # 1024 aggregated positions
LH = L // 2                # 2048 query positions per core
BN_EPS = 1e-5
N_CORES = 8

# Schraudolph exp-trick constants, int16/bfloat16 variant (trunc/floor):
# i16 = trunc(S * 2^7*log2e/8 + (127*2^7 - bias)); i16 bits read as bf16
# give exp(S/8) within ~3%. Folds the 1/sqrt(E)=1/8 score scale.
TRICK_A = 128.0 * 1.4426950408889634 * 0.125
TRICK_B = float(127 << 7) - 366400.0 / 65536.0

_CACHE = {}


def _build_program():
    import concourse.bass as bass
    import concourse.bacc as bacc
    import concourse.tile as tile
    from concourse import mybir
    from contextlib import ExitStack

    dt = mybir.dt
    f32 = dt.float32
    f32r = dt.float32r
    bf16 = dt.bfloat16
    i16 = dt.int16
    AF = mybir.ActivationFunctionType
    Alu = mybir.AluOpType

    nc = bacc.Bacc("TRN2", debug=False, num_devices=N_CORES)

    xf_d = nc.dram_tensor("x_full", [C, L], bf16, kind="ExternalInput")
    wqt_d = nc.dram_tensor("wqt", [C, C], bf16, kind="ExternalInput")
    wkt_d = nc.dram_tensor("wkt", [C, C], bf16, kind="ExternalInput")
    wvt_d = nc.dram_tensor("wvt", [C, C], bf16, kind="ExternalInput")
    wot_d = nc.dram_tensor("wot", [C, C], bf16, kind="ExternalInput")
    wat_d = nc.dram_tensor("wat", [C, C], bf16, kind="ExternalInput")
    # rows: bq, bk, t(bn-folded xa bias), bo
    bp_d = nc.dram_tensor("biasp", [4, C], f32, kind="ExternalInput")
    bv_d = nc.dram_tensor("bv", [C], f32, kind="ExternalInput")
    out_d = nc.dram_tensor("out", [C, LH], f32, kind="ExternalOutput")

    # Each exp chunk is split by columns: ACT runs real Exp on the first
    # EXP_ACT_COLS, DVE runs the int16 Schraudolph trick on the rest, in
    # parallel -- chunk latency ~0.71us < PE's 0.85us per-chunk appetite,
    # so PE (not the exp engines) paces the pipeline.
    EXP_ACT_COLS = 768

    with tile.TileContext(nc) as tc, ExitStack() as ctx:
        pp = ctx.enter_context(tc.tile_pool(name="persist", bufs=1))
        scr_w = ctx.enter_context(tc.tile_pool(name="scr_w", bufs=1))
        scr_p = ctx.enter_context(tc.tile_pool(name="scr_p", bufs=3))
        at_pool = ctx.enter_context(tc.tile_pool(name="at", bufs=6))
        oa_pool = ctx.enter_context(tc.tile_pool(name="oa", bufs=2))
        outp = ctx.enter_context(tc.tile_pool(name="outp", bufs=3))
        r_pool = ctx.enter_context(tc.tile_pool(name="rp", bufs=2))
        R_pool = ctx.enter_context(tc.tile_pool(name="Rp", bufs=3))

        ps_s = ctx.enter_context(tc.tile_pool(name="ps_s", bufs=3, space="PSUM"))
        ps_o = ctx.enter_context(tc.tile_pool(name="ps_o", bufs=2, space="PSUM"))

        # ---- persistent tiles ----
        xf = [pp.tile([128, L], bf16, name=f"xf{ct}", tag=f"xf{ct}")
              for ct in range(2)]
        q_r = [pp.tile([128, LH], bf16, name=f"qr{ct}", tag=f"qr{ct}")
               for ct in range(2)]
        k_r = [pp.tile([128, L2], bf16, name=f"kr{ct}", tag=f"kr{ct}")
               for ct in range(2)]
        xa_r = [pp.tile([128, L2], bf16, name=f"xar{ct}", tag=f"xar{ct}")
                for ct in range(2)]
        p_r = [pp.tile([128, L2], bf16, name=f"pr{ct}", tag=f"pr{ct}")
               for ct in range(2)]
        # v'^T per m-tile: 4 heads x (64 cols + ones col), bf16
        v_r = [pp.tile([128, 4 * 65], bf16, name=f"vr{mt}", tag=f"vr{mt}")
               for mt in range(8)]
        bias_t = [pp.tile([128, 4], f32, name=f"bias{ct}", tag=f"bias{ct}")
                  for ct in range(2)]
        bvb = pp.tile([128, C], f32, name="bvb", tag="bvb")

        # ---- DMAs: the cost model serializes DMA transfers (~360 B/ns
        # aggregate), so everything goes on the SP queue in first-use order;
        # putting DMAs on the ACT queue head-of-line-blocks its sequencer.
        wt_dram = {"wqt": wqt_d, "wkt": wkt_d, "wvt": wvt_d, "wot": wot_d,
                   "wat": wat_d}
        w_f = {}

        def w_dma(wname):
            wf = scr_w.tile([128, 512], bf16, name=f"wf_{wname}",
                            tag=f"wf_{wname}")
            src = wt_dram[wname].ap().rearrange("(k p) o -> p k o", p=128)
            nc.sync.dma_start(wf[:].rearrange("p (k o) -> p k o", k=2), src)
            w_f[wname] = wf

        def x_dma(half, sub):
            for ct in range(2):
                c0 = half * 2048 + sub * 1024
                nc.sync.dma_start(
                    xf[ct][:, c0:c0 + 1024],
                    xf_d.ap()[ct * 128:(ct + 1) * 128, c0:c0 + 1024])

        for ct in range(2):
            nc.sync.dma_start(
                bias_t[ct][:], bp_d.ap().rearrange("b (k p) -> k p b", p=128)[ct])
        bv_f = r_pool.tile([1, C], f32, name="bv_f", tag="bv_f", bufs=1)
        nc.sync.dma_start(bv_f[:], bv_d.ap().rearrange("(a o) -> a o", a=1))
        w_dma("wqt")
        x_dma(0, 0)   # cols 0:1024, both ct tiles
        w_dma("wat")
        w_dma("wkt")
        w_dma("wvt")
        x_dma(0, 1)
        x_dma(1, 0)
        x_dma(1, 1)
        w_dma("wot")

        # ---- constants ----
        for mt in range(8):
            nc.gpsimd.memset(
                v_r[mt][:].rearrange("p (h e) -> p h e", e=65)[:, :, 64], 1.0)
        # pre-warm the ACT exp table during the idle prefix
        warm = scr_w.tile([1, 8], f32, name="warm", tag="warm")
        ones_f = scr_w.tile([1, 8], f32, name="ones_f", tag="ones_f")
        nc.gpsimd.memset(ones_f[:], 1.0)
        nc.scalar.activation(warm[:], ones_f[:], AF.Exp, scale=1.0)
        # bv broadcast to all partitions (for the v-drain fused bias)
        nc.gpsimd.partition_broadcast(bvb[:], bv_f[:], channels=128)

        def w_block(wname, cch, ct_out):
            # lhsT block [c_in 128, c_out 128] for chunk cch, out tile ct_out
            return w_f[wname][:, cch * 256 + ct_out * 128:
                              cch * 256 + ct_out * 128 + 128]

        # ---- pool quadrants: p = avg4 + max4 ----
        def pool_quadrant(mc, ct, eng, sub=None):
            c0, cw = mc * 2048, 2048
            s0, sw = mc * 512, 512
            if sub is not None:
                c0, cw = c0 + sub * 1024, 1024
                s0, sw = s0 + sub * 256, 256
            xv = xf[ct][:, c0:c0 + cw].rearrange("p (m g) -> p m g", g=4)
            a1 = scr_p.tile([128, 512], f32, name="pa1", tag="pa1")
            a2 = scr_p.tile([128, 512], f32, name="pa2", tag="pa2")
            m1 = scr_p.tile([128, 512], f32, name="pm1", tag="pm1")
            m2 = scr_p.tile([128, 512], f32, name="pm2", tag="pm2")
            eng.tensor_tensor(a1[:, 0:sw], xv[:, :, 0], xv[:, :, 1], Alu.add)
            eng.tensor_tensor(a2[:, 0:sw], xv[:, :, 2], xv[:, :, 3], Alu.add)
            eng.tensor_tensor(m1[:, 0:sw], xv[:, :, 0], xv[:, :, 1], Alu.max)
            eng.tensor_tensor(m2[:, 0:sw], xv[:, :, 2], xv[:, :, 3], Alu.max)
            eng.tensor_tensor(a1[:, 0:sw], a1[:, 0:sw], a2[:, 0:sw], Alu.add)
            eng.tensor_tensor(m1[:, 0:sw], m1[:, 0:sw], m2[:, 0:sw], Alu.max)
            eng.scalar_tensor_tensor(
                p_r[ct][:, s0:s0 + sw], a1[:, 0:sw], 0.25, m1[:, 0:sw],
                Alu.mult, Alu.add)

        # ---- projection chunk helpers ----
        def proj_chunk(wname, src, dst, bias_col, c0, cw, eng):
            for ct_out in range(2):
                ps = ps_s.tile([128, cw], f32, name="ps_s", tag="ps_s")
                for cch in range(2):
                    nc.tensor.matmul(
                        ps[:], w_block(wname, cch, ct_out),
                        src[cch][:, c0:c0 + cw],
                        start=(cch == 0), stop=(cch == 1))
                if eng is nc.scalar:
                    nc.scalar.add(dst[ct_out][:, c0:c0 + cw],
                                  ps[:], bias_t[ct_out][:, bias_col:bias_col + 1])
                else:
                    eng.tensor_scalar(
                        dst[ct_out][:, c0:c0 + cw], ps[:],
                        bias_t[ct_out][:, bias_col:bias_col + 1], None, Alu.add)

        def q_chunk(lcq, eng):
            for ct_out in range(2):
                ps = ps_s.tile([128, 512], f32, name="ps_s", tag="ps_s")
                for cch in range(2):
                    nc.tensor.matmul(
                        ps[:], w_block("wqt", cch, ct_out),
                        xf[cch][:, lcq * 512:(lcq + 1) * 512],
                        start=(cch == 0), stop=(cch == 1))
                if eng is nc.scalar:
                    nc.scalar.add(q_r[ct_out][:, lcq * 512:(lcq + 1) * 512],
                                  ps[:], bias_t[ct_out][:, 0:1])
                else:
                    eng.tensor_scalar(
                        q_r[ct_out][:, lcq * 512:(lcq + 1) * 512], ps[:],
                        bias_t[ct_out][:, 0:1], None, Alu.add)

        def v_block(mt, drain_eng, vpool=None):
            vpool = vpool or ps_o
            tag = "ps_o" if vpool is ps_o else "ps_s"
            pv = vpool.tile([128, C], f32, name="ps_v", tag=tag)
            for cch in range(2):
                nc.tensor.matmul(
                    pv[:], xa_r[cch][:, mt * 128:(mt + 1) * 128],
                    w_f["wvt"][:, cch * 256:(cch + 1) * 256],
                    start=(cch == 0), stop=(cch == 1))
            vv = v_r[mt][:].rearrange("p (h e) -> p h e", e=65)
            # fused +bv via the broadcast bias tile
            drain_eng.scalar_tensor_tensor(
                vv[:, :, 0:64], pv[:].rearrange("p (h e) -> p h e", e=64),
                1.0, bvb[:].rearrange("p (h e) -> p h e", e=64),
                Alu.mult, Alu.add)

        # ---- key-block groups: 256 keys each, gated on one x DMA pair.
        # pool both ct tiles -> xa block -> k block -> two v blocks ----
        def blkgrp(b, drain_eng):
            mc, sub = divmod(b, 2)
            pool_quadrant(mc, 0, nc.vector, sub=sub)
            pool_quadrant(mc, 1, nc.vector, sub=sub)
            proj_chunk("wat", p_r, xa_r, 2, b * 256, 256, drain_eng)
            proj_chunk("wkt", xa_r, k_r, 1, b * 256, 256, drain_eng)
            for mt in (2 * b, 2 * b + 1):
                v_block(mt, nc.vector, vpool=ps_s)

        # ---- prefix: q0/q1 + first two key-block groups (x half 0) ----
        q_chunk(0, nc.scalar)
        blkgrp(0, nc.scalar)
        q_chunk(1, nc.scalar)
        blkgrp(1, nc.scalar)
        q_chunk(2, nc.vector)
        q_chunk(3, nc.vector)

        # ---- attention: o-matmuls lag exp by one m-tile; the previous
        # iteration's softmax-normalize and Wo conv are emitted inside the
        # next iteration's S/exp stream so they overlap it ----
        oa_tiles = {}

        def norm_prev(state):
            lc, hp, po = state
            oa = oa_tiles[lc]
            for h2 in range(2):
                r_t = r_pool.tile([1, 512], f32, name="r", tag="r")
                nc.vector.reciprocal(r_t[:], po[h2][64:65, :])
                R_t = R_pool.tile([64, 512], f32, name="R", tag="R")
                nc.gpsimd.partition_broadcast(R_t[:], r_t[:], channels=64)
                nc.vector.tensor_tensor(
                    oa[hp][h2 * 64:(h2 + 1) * 64, :], po[h2][0:64, :],
                    R_t[:], Alu.mult)

        def wo_prev(state):
            lc, hp, po = state
            if hp != 1:
                return
            oa = oa_tiles[lc]
            for ct_out in range(2):
                psW = ps_s.tile([128, 512], f32, name="ps_s", tag="ps_s")
                for cch in range(2):
                    nc.tensor.matmul(
                        psW[:], w_block("wot", cch, ct_out), oa[cch][:],
                        start=(cch == 0), stop=(cch == 1))
                out_t = outp.tile([128, 512], f32, name="out", tag="out")
                nc.scalar.add(out_t[:], psW[:], bias_t[ct_out][:, 3:4])
                nc.sync.dma_start(
                    out_d.ap()[ct_out * 128:(ct_out + 1) * 128,
                               lc * 512:(lc + 1) * 512], out_t[:])
            del oa_tiles[lc]

        # pending o-matmul FIFO: one pair popped per (S, exp) step, crossing
        # iteration boundaries so PE never waits on the last exp of an iter
        pending = []
        it_idx = [0]

        def emit_iter(lc, hp, prev_state, hooks=None):
            it = it_idx[0]
            it_idx[0] += 1
            if hp == 0:
                oa_tiles[lc] = [
                    oa_pool.tile([128, 512], bf16, name=f"oa{ct}",
                                 tag=f"oa{ct}") for ct in range(2)]
            po = [ps_o.tile([65, 512], f32, name="ps_o", tag="ps_o")
                  for _ in range(2)]

            def make_o(mt, at_ap):
                def emit():
                    for h2 in range(2):
                        h = 2 * hp + h2
                        nc.tensor.matmul(
                            po[h2][:], v_r[mt][:, h * 65:h * 65 + 65],
                            at_ap[:, h2 * 512:(h2 + 1) * 512],
                            start=(mt == 0), stop=(mt == 7))
                return emit

            for mt in range(8):
                if hooks and mt in hooks:
                    hooks[mt]()
                ps = ps_s.tile([128, L2], f32, name="ps_s", tag="ps_s")
                for h2 in range(2):
                    nc.tensor.matmul(
                        ps[:, h2 * 512:(h2 + 1) * 512],
                        k_r[hp][h2 * 64:(h2 + 1) * 64, mt * 128:(mt + 1) * 128],
                        q_r[hp][h2 * 64:(h2 + 1) * 64, lc * 512:(lc + 1) * 512],
                        start=True, stop=True)
                # iter 0: DVE is busy streaming pool blocks (and its queue
                # would head-of-line block on DMA-gated ops), so all-ACT
                ca = 1024 if it == 0 else EXP_ACT_COLS
                at = at_pool.tile([128, 1024], bf16, name="at", tag="at")
                nc.scalar.activation(at[:, 0:ca], ps[:, 0:ca], AF.Exp,
                                     scale=0.125)
                if ca < 1024:
                    nc.vector.tensor_scalar(at[:].bitcast(i16)[:, ca:1024],
                                            ps[:, ca:1024], TRICK_A, TRICK_B,
                                            Alu.mult, Alu.add)
                at_ap = at[:]
                pending.append(make_o(mt, at_ap))
                # with the deeper o-FIFO, the previous iteration's last
                # o-matmul is popped during step mt1, so its normalize may
                # be emitted no earlier than mt2 (else it misses mt7)
                if mt == 2 and prev_state is not None:
                    norm_prev(prev_state)
                if mt == 5 and prev_state is not None:
                    wo_prev(prev_state)
                if len(pending) >= 3:
                    pending.pop(0)()
            return (lc, hp, po)

        # iteration (0,0): key-block groups 2/3 (x half 1) stream in mid-iter
        state = emit_iter(0, 0, None, hooks={
            3: lambda: blkgrp(2, nc.vector),
            5: lambda: blkgrp(3, nc.vector),
        })
        for lc, hp in [(0, 1), (1, 0), (1, 1), (2, 0), (2, 1), (3, 0), (3, 1)]:
            state = emit_iter(lc, hp, state)
        while pending:
            pending.pop(0)()
        norm_prev(state)
        wo_prev(state)

    nc.compile()
    return nc


def _get_program():
    if "nc" not in _CACHE:
        _CACHE["nc"] = _build_program()
    return _CACHE["nc"]


def kernel(x, Wq, bq, Wk, bk, Wv, bv, Wo, bo, Wa,
           g1, b1, m1, v1, g2, b2, m2, v2):
    import ml_dtypes
    from concourse import bass_utils

    nc = _get_program()
    bf = ml_dtypes.bfloat16

    x = np.asarray(x, dtype=np.float32).astype(bf)
    # fold both eval-mode BNs into a per-channel affine: xa = s*(Wa@p) + t
    s1 = np.asarray(g1) / np.sqrt(np.asarray(v1) + BN_EPS)
    t1 = np.asarray(b1) - np.asarray(m1) * s1
    s2 = np.asarray(g2) / np.sqrt(np.asarray(v2) + BN_EPS)
    t2 = np.asarray(b2) - np.asarray(m2) * s2
    s = (s1 * s2).astype(np.float32)
    t = (t1 * s2 + t2).astype(np.float32)

    wat = (np.asarray(Wa) * s[:, None]).astype(np.float32).T.astype(bf)
    wqt = np.asarray(Wq, dtype=np.float32).T.astype(bf)
    wkt = np.asarray(Wk, dtype=np.float32).T.astype(bf)
    wvt = np.asarray(Wv, dtype=np.float32).T.astype(bf)
    wot = np.asarray(Wo, dtype=np.float32).T.astype(bf)
    biasp = np.stack([np.asarray(bq), np.asarray(bk), t,
                      np.asarray(bo)]).astype(np.float32)
    bvv = np.asarray(bv, dtype=np.float32)

    shared = {"wqt": wqt, "wkt": wkt, "wvt": wvt, "wot": wot, "wat": wat,
              "biasp": biasp, "bv": bvv}
    in_maps = []
    for c in range(N_CORES):
        n, half = c // 2, c % 2
        m = dict(shared)
        xs = x[n]
        if half == 0:
            m["x_full"] = np.ascontiguousarray(xs)
        else:
            # core's own query half first; key order is irrelevant
            # (pool windows intact, attention permutation-invariant)
            m["x_full"] = np.concatenate([xs[:, LH:], xs[:, :LH]], axis=1)
        in_maps.append(m)

    res = bass_utils.run_bass_kernel_spmd(nc, in_maps,
                                          core_ids=list(range(N_CORES)))
    out = np.empty((N, C, L), np.float32)
    for c in range(N_CORES):
        n, half = c // 2, c % 2
        out[n][:, half * LH:(half + 1) * LH] = res.results[c]["out"]
    return out


# revision 49
# speedup vs baseline: 1.0987x; 1.0613x over previous
"""TRN2 Bass kernel for nn_AttentionBlock (N=4, C=256, L=4096, 4 heads, AGGR=4).

Sharding: 8 cores = (batch n, L-half). Core c handles n=c//2, query positions
l in [half*2048, (half+1)*2048). Each core computes k/v from the full
aggregated sequence of its batch (L2=1024) and produces the full output slice
out[n][:, l_half] -- no cross-core reduction needed.

The host hands each core x[n] with columns PERMUTED so the core's own query
half comes first (attention is permutation-invariant over key positions, and
the 4-wide pooling windows stay intact), so the query slice is a static
[:, 0:2048] view and is available as soon as the first DMA half lands.

Cost-model shape: every engine instruction costs (free-dim cols) x cycle_t;
PE matmuls cost (out free cols) x 0.417ns regardless of contraction width.
The kernel is PE-bound (~70us of matmul cols), so softmax exp -- the other
big consumer (64 tiles x 1024 cols) -- is split across three engines so none
exceeds PE: ACT runs real Exp; Pool and DVE run a Schraudolph bit-trick exp
(i32 = trunc(S*2^23*log2e*0.125 + bias), bitcast as f32 ~ exp(S/8) within
3%), writing int32 tiles the o-matmul consumes as float32r. f32->f32r
bitcast views avoid all weight/x re-typing copies.
"""

import numpy as np

N, C, L = 4, 256, 4096
HEAD_DIM = 64
H = C // HEAD_DIM          # 4 heads
AGGR = 4
L2 = L // AGGR             # 1024 aggregated positions
LH = L // 2                # 2048 query positions per core
BN_EPS = 1e-5
N_CORES = 8

# Schraudolph exp-trick constants, int16/bfloat16 variant (trunc/floor):
# i16 = trunc(S * 2^7*log2e/8 + (127*2^7 - bias)); i16 bits read as bf16
# give exp(S/8) within ~3%. Folds the 1/sqrt(E)=1/8 score scale.
TRICK_A = 128.0 * 1.4426950408889634 * 0.125
TRICK_B = float(127 << 7) - 366400.0 / 65536.0

# Phase-block permutation: within each 1024-col block, kernel col 256*j + m
# holds original col 4*m + j (phase-major), so the 4 pooling phases are
# contiguous runs. Queries travel permuted through the kernel; the host
# un-permutes the output columns. PHASE_PERM[p] = original col at kernel col p.
_tmp = np.arange(1024).reshape(256, 4).T.reshape(-1)
PHASE_PERM = np.concatenate([1024 * b + _tmp for b in range(4)])

_CACHE = {}


def _build_program():
    import concourse.bass as bass
    import concourse.bacc as bacc
    import concourse.tile as tile
    from concourse import mybir
    from contextlib import ExitStack

    dt = mybir.dt
    f32 = dt.float32
    f32r = dt.float32r
    bf16 = dt.bfloat16
    i16 = dt.int16
    AF = mybir.ActivationFunctionType
    Alu = mybir.AluOpType

    nc = bacc.Bacc("TRN2", debug=False, num_devices=N_CORES)

    xf_d = nc.dram_tensor("x_full", [C, L], bf16, kind="ExternalInput")
    wqt_d = nc.dram_tensor("wqt", [C, C], bf16, kind="ExternalInput")
    wkt_d = nc.dram_tensor("wkt", [C, C], bf16, kind="ExternalInput")
    wvt_d = nc.dram_tensor("wvt", [C, C], bf16, kind="ExternalInput")
    wot_d = nc.dram_tensor("wot", [C, C], bf16, kind="ExternalInput")
    wat_d = nc.dram_tensor("wat", [C, C], bf16, kind="ExternalInput")
    # rows: bq, bk, t(bn-folded xa bias), bo
    bp_d = nc.dram_tensor("biasp", [4, C], f32, kind="ExternalInput")
    bv_d = nc.dram_tensor("bv", [C], f32, kind="ExternalInput")
    out_d = nc.dram_tensor("out", [C, LH], f32, kind="ExternalOutput")

    # Each exp chunk is split by columns: ACT runs real Exp on the first
    # EXP_ACT_COLS, DVE runs the int16 Schraudolph trick on the rest, in
    # parallel -- chunk latency ~0.71us < PE's 0.85us per-chunk appetite,
    # so PE (not the exp engines) paces the pipeline.
    EXP_ACT_COLS = 768

    with tile.TileContext(nc) as tc, ExitStack() as ctx:
        pp = ctx.enter_context(tc.tile_pool(name="persist", bufs=1))
        scr_w = ctx.enter_context(tc.tile_pool(name="scr_w", bufs=1))
        scr_p = ctx.enter_context(tc.tile_pool(name="scr_p", bufs=3))
        at_pool = ctx.enter_context(tc.tile_pool(name="at", bufs=6))
        oa_pool = ctx.enter_context(tc.tile_pool(name="oa", bufs=2))
        outp = ctx.enter_context(tc.tile_pool(name="outp", bufs=3))
        r_pool = ctx.enter_context(tc.tile_pool(name="rp", bufs=2))
        R_pool = ctx.enter_context(tc.tile_pool(name="Rp", bufs=3))

        ps_s = ctx.enter_context(tc.tile_pool(name="ps_s", bufs=3, space="PSUM"))
        ps_o = ctx.enter_context(tc.tile_pool(name="ps_o", bufs=2, space="PSUM"))

        # ---- persistent tiles ----
        xf = [pp.tile([128, L], bf16, name=f"xf{ct}", tag=f"xf{ct}")
              for ct in range(2)]
        q_r = [pp.tile([128, LH], bf16, name=f"qr{ct}", tag=f"qr{ct}")
               for ct in range(2)]
        k_r = [pp.tile([128, L2], bf16, name=f"kr{ct}", tag=f"kr{ct}")
               for ct in range(2)]
        xa_r = [pp.tile([128, L2], bf16, name=f"xar{ct}", tag=f"xar{ct}")
                for ct in range(2)]
        p_r = [pp.tile([128, L2], bf16, name=f"pr{ct}", tag=f"pr{ct}")
               for ct in range(2)]
        # v'^T per m-tile: 4 heads x (64 cols + ones col), bf16
        v_r = [pp.tile([128, 4 * 65], bf16, name=f"vr{mt}", tag=f"vr{mt}")
               for mt in range(8)]
        bias_t = [pp.tile([128, 4], f32, name=f"bias{ct}", tag=f"bias{ct}")
                  for ct in range(2)]
        bvb = pp.tile([128, C], f32, name="bvb", tag="bvb")

        # ---- DMAs: the cost model serializes DMA transfers (~360 B/ns
        # aggregate), so everything goes on the SP queue in first-use order;
        # putting DMAs on the ACT queue head-of-line-blocks its sequencer.
        wt_dram = {"wqt": wqt_d, "wkt": wkt_d, "wvt": wvt_d, "wot": wot_d,
                   "wat": wat_d}
        w_f = {}

        def w_dma(wname):
            wf = scr_w.tile([128, 512], bf16, name=f"wf_{wname}",
                            tag=f"wf_{wname}")
            src = wt_dram[wname].ap().rearrange("(k p) o -> p k o", p=128)
            nc.sync.dma_start(wf[:].rearrange("p (k o) -> p k o", k=2), src)
            w_f[wname] = wf

        def x_dma(half, sub):
            for ct in range(2):
                c0 = half * 2048 + sub * 1024
                nc.sync.dma_start(
                    xf[ct][:, c0:c0 + 1024],
                    xf_d.ap()[ct * 128:(ct + 1) * 128, c0:c0 + 1024])

        for ct in range(2):
            nc.sync.dma_start(
                bias_t[ct][:], bp_d.ap().rearrange("b (k p) -> k p b", p=128)[ct])
        bv_f = r_pool.tile([1, C], f32, name="bv_f", tag="bv_f", bufs=1)
        nc.sync.dma_start(bv_f[:], bv_d.ap().rearrange("(a o) -> a o", a=1))
        w_dma("wqt")
        x_dma(0, 0)   # cols 0:1024, both ct tiles
        w_dma("wat")
        w_dma("wkt")
        w_dma("wvt")
        x_dma(0, 1)
        x_dma(1, 0)
        x_dma(1, 1)
        w_dma("wot")

        # ---- constants ----
        for mt in range(8):
            nc.gpsimd.memset(
                v_r[mt][:].rearrange("p (h e) -> p h e", e=65)[:, :, 64], 1.0)
        # pre-warm the ACT exp table during the idle prefix
        warm = scr_w.tile([1, 8], f32, name="warm", tag="warm")
        ones_f = scr_w.tile([1, 8], f32, name="ones_f", tag="ones_f")
        nc.gpsimd.memset(ones_f[:], 1.0)
        nc.scalar.activation(warm[:], ones_f[:], AF.Exp, scale=1.0)
        # bv broadcast to all partitions (for the v-drain fused bias)
        nc.gpsimd.partition_broadcast(bvb[:], bv_f[:], channels=128)

        def w_block(wname, cch, ct_out):
            # lhsT block [c_in 128, c_out 128] for chunk cch, out tile ct_out
            return w_f[wname][:, cch * 256 + ct_out * 128:
                              cch * 256 + ct_out * 128 + 128]

        # ---- pool: p = avg4 + max4 over phase-blocked x. The host lays out
        # each 1024-col block as [ph0|ph1|ph2|ph3] (x[:, 4m+j] at phase j,
        # window m), so every operand is a contiguous bf16 run and the DVE
        # runs at its 2x packed rate. Queries are un-permuted on the host.
        def pool_block(b, ct, eng):
            xv = xf[ct][:, b * 1024:(b + 1) * 1024].rearrange(
                "p (j m) -> p j m", j=4)
            a1 = scr_p.tile([128, 256], bf16, name="pa1", tag="pa1")
            a2 = scr_p.tile([128, 256], bf16, name="pa2", tag="pa2")
            m1 = scr_p.tile([128, 256], bf16, name="pm1", tag="pm1")
            m2 = scr_p.tile([128, 256], bf16, name="pm2", tag="pm2")
            eng.tensor_tensor(a1[:], xv[:, 0], xv[:, 1], Alu.add)
            eng.tensor_tensor(a2[:], xv[:, 2], xv[:, 3], Alu.add)
            eng.tensor_tensor(m1[:], xv[:, 0], xv[:, 1], Alu.max)
            eng.tensor_tensor(m2[:], xv[:, 2], xv[:, 3], Alu.max)
            eng.tensor_tensor(a1[:], a1[:], a2[:], Alu.add)
            eng.tensor_tensor(m1[:], m1[:], m2[:], Alu.max)
            eng.scalar_tensor_tensor(
                p_r[ct][:, b * 256:(b + 1) * 256], a1[:], 0.25, m1[:],
                Alu.mult, Alu.add)

        # ---- projection chunk helpers ----
        def proj_chunk(wname, src, dst, bias_col, c0, cw, eng):
            for ct_out in range(2):
                ps = ps_s.tile([128, cw], f32, name="ps_s", tag="ps_s")
                for cch in range(2):
                    nc.tensor.matmul(
                        ps[:], w_block(wname, cch, ct_out),
                        src[cch][:, c0:c0 + cw],
                        start=(cch == 0), stop=(cch == 1))
                if eng is nc.scalar:
                    nc.scalar.add(dst[ct_out][:, c0:c0 + cw],
                                  ps[:], bias_t[ct_out][:, bias_col:bias_col + 1])
                else:
                    eng.tensor_scalar(
                        dst[ct_out][:, c0:c0 + cw], ps[:],
                        bias_t[ct_out][:, bias_col:bias_col + 1], None, Alu.add)

        def q_chunk(lcq, eng):
            for ct_out in range(2):
                ps = ps_s.tile([128, 512], f32, name="ps_s", tag="ps_s")
                for cch in range(2):
                    nc.tensor.matmul(
                        ps[:], w_block("wqt", cch, ct_out),
                        xf[cch][:, lcq * 512:(lcq + 1) * 512],
                        start=(cch == 0), stop=(cch == 1))
                if eng is nc.scalar:
                    nc.scalar.add(q_r[ct_out][:, lcq * 512:(lcq + 1) * 512],
                                  ps[:], bias_t[ct_out][:, 0:1])
                else:
                    eng.tensor_scalar(
                        q_r[ct_out][:, lcq * 512:(lcq + 1) * 512], ps[:],
                        bias_t[ct_out][:, 0:1], None, Alu.add)

        def v_block(mt, drain_eng, vpool=None):
            vpool = vpool or ps_o
            tag = "ps_o" if vpool is ps_o else "ps_s"
            pv = vpool.tile([128, C], f32, name="ps_v", tag=tag)
            for cch in range(2):
                nc.tensor.matmul(
                    pv[:], xa_r[cch][:, mt * 128:(mt + 1) * 128],
                    w_f["wvt"][:, cch * 256:(cch + 1) * 256],
                    start=(cch == 0), stop=(cch == 1))
            vv = v_r[mt][:].rearrange("p (h e) -> p h e", e=65)
            # fused +bv via the broadcast bias tile
            drain_eng.scalar_tensor_tensor(
                vv[:, :, 0:64], pv[:].rearrange("p (h e) -> p h e", e=64),
                1.0, bvb[:].rearrange("p (h e) -> p h e", e=64),
                Alu.mult, Alu.add)

        # ---- key-block groups: 256 keys each, gated on one x DMA pair.
        # pool both ct tiles -> xa block -> k block -> two v blocks ----
        def blkgrp(b, drain_eng):
            pool_block(b, 0, nc.vector)
            pool_block(b, 1, nc.vector)
            proj_chunk("wat", p_r, xa_r, 2, b * 256, 256, drain_eng)
            proj_chunk("wkt", xa_r, k_r, 1, b * 256, 256, drain_eng)
            for mt in (2 * b, 2 * b + 1):
                v_block(mt, nc.vector, vpool=ps_s)

        # ---- prefix: q0/q1 + first two key-block groups (x half 0) ----
        q_chunk(0, nc.scalar)
        blkgrp(0, nc.scalar)
        q_chunk(1, nc.scalar)
        blkgrp(1, nc.scalar)
        q_chunk(2, nc.vector)
        q_chunk(3, nc.vector)

        # ---- attention: o-matmuls lag exp by one m-tile; the previous
        # iteration's softmax-normalize and Wo conv are emitted inside the
        # next iteration's S/exp stream so they overlap it ----
        oa_tiles = {}

        def norm_prev(state):
            lc, hp, po = state
            oa = oa_tiles[lc]
            for h2 in range(2):
                r_t = r_pool.tile([1, 512], f32, name="r", tag="r")
                nc.vector.reciprocal(r_t[:], po[h2][64:65, :])
                R_t = R_pool.tile([64, 512], f32, name="R", tag="R")
                nc.gpsimd.partition_broadcast(R_t[:], r_t[:], channels=64)
                nc.vector.tensor_tensor(
                    oa[hp][h2 * 64:(h2 + 1) * 64, :], po[h2][0:64, :],
                    R_t[:], Alu.mult)

        def wo_prev(state):
            lc, hp, po = state
            if hp != 1:
                return
            oa = oa_tiles[lc]
            for ct_out in range(2):
                psW = ps_s.tile([128, 512], f32, name="ps_s", tag="ps_s")
                for cch in range(2):
                    nc.tensor.matmul(
                        psW[:], w_block("wot", cch, ct_out), oa[cch][:],
                        start=(cch == 0), stop=(cch == 1))
                out_t = outp.tile([128, 512], f32, name="out", tag="out")
                nc.vector.tensor_scalar(out_t[:], psW[:],
                                        bias_t[ct_out][:, 3:4], None, Alu.add)
                nc.sync.dma_start(
                    out_d.ap()[ct_out * 128:(ct_out + 1) * 128,
                               lc * 512:(lc + 1) * 512], out_t[:])
            del oa_tiles[lc]

        # pending o-matmul FIFO: one pair popped per (S, exp) step, crossing
        # iteration boundaries so PE never waits on the last exp of an iter
        pending = []
        it_idx = [0]

        def emit_iter(lc, hp, prev_state, hooks=None):
            it = it_idx[0]
            it_idx[0] += 1
            if hp == 0:
                oa_tiles[lc] = [
                    oa_pool.tile([128, 512], bf16, name=f"oa{ct}",
                                 tag=f"oa{ct}") for ct in range(2)]
            po = [ps_o.tile([65, 512], f32, name="ps_o", tag="ps_o")
                  for _ in range(2)]

            def make_o(mt, at_ap):
                def emit():
                    for h2 in range(2):
                        h = 2 * hp + h2
                        nc.tensor.matmul(
                            po[h2][:], v_r[mt][:, h * 65:h * 65 + 65],
                            at_ap[:, h2 * 512:(h2 + 1) * 512],
                            start=(mt == 0), stop=(mt == 7))
                return emit

            for mt in range(8):
                if hooks and mt in hooks:
                    hooks[mt]()
                ps = ps_s.tile([128, L2], f32, name="ps_s", tag="ps_s")
                for h2 in range(2):
                    nc.tensor.matmul(
                        ps[:, h2 * 512:(h2 + 1) * 512],
                        k_r[hp][h2 * 64:(h2 + 1) * 64, mt * 128:(mt + 1) * 128],
                        q_r[hp][h2 * 64:(h2 + 1) * 64, lc * 512:(lc + 1) * 512],
                        start=True, stop=True)
                # iter 0: DVE is busy streaming pool blocks (and its queue
                # would head-of-line block on DMA-gated ops), so all-ACT
                ca = 1024 if it == 0 else EXP_ACT_COLS
                at = at_pool.tile([128, 1024], bf16, name="at", tag="at")
                nc.scalar.activation(at[:, 0:ca], ps[:, 0:ca], AF.Exp,
                                     scale=0.125)
                if ca < 1024:
                    nc.vector.tensor_scalar(at[:].bitcast(i16)[:, ca:1024],
                                            ps[:, ca:1024], TRICK_A, TRICK_B,
                                            Alu.mult, Alu.add)
                at_ap = at[:]
                pending.append(make_o(mt, at_ap))
                # with the deeper o-FIFO, the previous iteration's last
                # o-matmul is popped during step mt1, so its normalize may
                # be emitted no earlier than mt2 (else it misses mt7)
                if mt == 2 and prev_state is not None:
                    norm_prev(prev_state)
                if mt == 5 and prev_state is not None:
                    wo_prev(prev_state)
                if len(pending) >= 3:
                    pending.pop(0)()
            return (lc, hp, po)

        # iteration (0,0): key-block groups 2/3 (x half 1) stream in mid-iter
        state = emit_iter(0, 0, None, hooks={
            3: lambda: blkgrp(2, nc.vector),
            5: lambda: blkgrp(3, nc.vector),
        })
        for lc, hp in [(0, 1), (1, 0), (1, 1), (2, 0), (2, 1), (3, 0), (3, 1)]:
            state = emit_iter(lc, hp, state)
        while pending:
            pending.pop(0)()
        norm_prev(state)
        wo_prev(state)

    nc.compile()
    return nc


def _get_program():
    if "nc" not in _CACHE:
        _CACHE["nc"] = _build_program()
    return _CACHE["nc"]


def kernel(x, Wq, bq, Wk, bk, Wv, bv, Wo, bo, Wa,
           g1, b1, m1, v1, g2, b2, m2, v2):
    import ml_dtypes
    from concourse import bass_utils

    nc = _get_program()
    bf = ml_dtypes.bfloat16

    x = np.asarray(x, dtype=np.float32).astype(bf)
    # fold both eval-mode BNs into a per-channel affine: xa = s*(Wa@p) + t
    s1 = np.asarray(g1) / np.sqrt(np.asarray(v1) + BN_EPS)
    t1 = np.asarray(b1) - np.asarray(m1) * s1
    s2 = np.asarray(g2) / np.sqrt(np.asarray(v2) + BN_EPS)
    t2 = np.asarray(b2) - np.asarray(m2) * s2
    s = (s1 * s2).astype(np.float32)
    t = (t1 * s2 + t2).astype(np.float32)

    wat = (np.asarray(Wa) * s[:, None]).astype(np.float32).T.astype(bf)
    wqt = np.asarray(Wq, dtype=np.float32).T.astype(bf)
    wkt = np.asarray(Wk, dtype=np.float32).T.astype(bf)
    wvt = np.asarray(Wv, dtype=np.float32).T.astype(bf)
    wot = np.asarray(Wo, dtype=np.float32).T.astype(bf)
    biasp = np.stack([np.asarray(bq), np.asarray(bk), t,
                      np.asarray(bo)]).astype(np.float32)
    bvv = np.asarray(bv, dtype=np.float32)

    shared = {"wqt": wqt, "wkt": wkt, "wvt": wvt, "wot": wot, "wat": wat,
              "biasp": biasp, "bv": bvv}
    in_maps = []
    for c in range(N_CORES):
        n, half = c // 2, c % 2
        m = dict(shared)
        xs = x[n]
        if half == 1:
            # core's own query half first; key order is irrelevant
            # (pool windows intact, attention permutation-invariant)
            xs = np.concatenate([xs[:, LH:], xs[:, :LH]], axis=1)
        m["x_full"] = np.ascontiguousarray(xs[:, PHASE_PERM])
        in_maps.append(m)

    res = bass_utils.run_bass_kernel_spmd(nc, in_maps,
                                          core_ids=list(range(N_CORES)))
    out = np.empty((N, C, L), np.float32)
    qp = PHASE_PERM[:LH]
    for c in range(N_CORES):
        n, half = c // 2, c % 2
        o = np.asarray(res.results[c]["out"])
        out[n][:, half * LH + qp] = o
    return out
